# revision 2
# baseline (speedup 1.0000x reference)
"""Multi-head attention (B=4, S=2048, D=1024, H=16, Dh=64) on 8 NeuronCores, v2.

Sharding: core c handles batch b=c//2 and head-group g=c%2 (8 heads = 4 pairs).
wq/wk/wv column-parallel, wo row-parallel; host sums the two bf16 partial
wo-products per batch and adds bo.

v2 changes vs baseline:
- reciprocal_approx_fast (51 ULP) instead of 3.35us reciprocal
- merged 2-head exp per i-tile: one ACT instr over [128, 2x(512-c0)] PSUM
- col-tiled PV (M=64 at col positions 0/64, concurrent) + separate den
  matmuls (M=1 at col 0/32, concurrent) -> head B lands at partitions
  64-127 so normalization writes aout directly (no partition-shift DMA)
- j-outer/p-inner attention; Wo contracts K=512 across all 4 pairs in
  PSUM; Wo matmuls drip-fed into the next j-chunk's i-loop; single bf16
  output partial per core (4MB instead of 32MB f32)
- host-side pre-arranged DRAM layouts -> contiguous multi-KB DMA packets
- QK bias on DVE tensor_scalar; V/Wo PSUM copies on GPSIMD; exp only on ACT
"""

import sys

sys.path.insert(0, "/opt/trn_rl_repo")

import ml_dtypes
import numpy as np

import concourse.bass as bass  # noqa: F401
import concourse.bacc as bacc
import concourse.tile as tile
import concourse.mybir as mybir
from concourse.bass_utils import run_bass_kernel_spmd

F32 = mybir.dt.float32
F32R = mybir.dt.float32r
BF16 = mybir.dt.bfloat16
AF = mybir.ActivationFunctionType
BF = ml_dtypes.bfloat16

B, S, D = 4, 2048, 1024
H, DH = 16, 64
HG = 8  # heads per core
DG = HG * DH  # 512 out-dims per core

_PROGRAM = None
LAST_RESULTS = None  # for test.py introspection


def _build_program(do_compile=True):
    nc = bacc.Bacc("TRN2", target_bir_lowering=False, debug=False)

    # pre-arranged inputs (see _make_in_maps for layouts)
    xq_c = nc.dram_tensor("xq_c", [2, 128, 8 * 1024], BF16, kind="ExternalInput")
    xk_c = nc.dram_tensor("xk_c", [2, 128, 8 * 1024], BF16, kind="ExternalInput")
    xv_c = nc.dram_tensor("xv_c", [16, 128, 8 * 128], BF16, kind="ExternalInput")
    wq_c = nc.dram_tensor("wq_c", [128, 8 * 512], BF16, kind="ExternalInput")
    wk_c = nc.dram_tensor("wk_c", [128, 8 * 512], BF16, kind="ExternalInput")
    wv_c = nc.dram_tensor("wv_c", [128, 8 * 512], BF16, kind="ExternalInput")
    wo_c = nc.dram_tensor("wo_c", [DG, D], BF16, kind="ExternalInput")
    bq_c = nc.dram_tensor("bq_c", [128, 4], F32, kind="ExternalInput")
    bk_c = nc.dram_tensor("bk_c", [128, 4], F32, kind="ExternalInput")
    bv_r = nc.dram_tensor("bv_r", [1, DG], BF16, kind="ExternalInput")
    ones_b = nc.dram_tensor("ones_b", [1, 128], BF16, kind="ExternalInput")
    ones_col = nc.dram_tensor("ones_col", [128, 1], BF16, kind="ExternalInput")
    onesel_d = nc.dram_tensor("onesel_d", [128, 128], F32R, kind="ExternalInput")
    rbones_d = nc.dram_tensor("rbones_d", [128, 512], F32R, kind="ExternalInput")
    mask01 = nc.dram_tensor("mask01", [128, 128], BF16, kind="ExternalInput")
    out_d = nc.dram_tensor("out0", [S, D], BF16, kind="ExternalOutput")

    with tile.TileContext(nc) as tc:
        with (
            nc.allow_low_precision(reason="bf16 attention pipeline"),
            tc.tile_pool(name="persist", bufs=1) as pers,
        ):
            # ---- persistent tiles ----
            qT = [pers.tile([128, S], BF16, name=f"qT{i}") for i in range(4)]
            kT = [pers.tile([128, S], BF16, name=f"kT{i}") for i in range(4)]
            vt = [pers.tile([128, DG], BF16, name=f"v{i}") for i in range(16)]
            aout = [pers.tile([128, S], BF16, name=f"ao{i}") for i in range(4)]
            wo_sb = [pers.tile([128, D], BF16, name=f"wo{c}") for c in range(4)]
            mask_sb = pers.tile([128, 128], BF16, name="mask01")
            ones_bv = pers.tile([1, 128], BF16, name="ones_bv")
            ones_w = pers.tile([128, 1], BF16, name="ones_w")
            bq_sb = pers.tile([128, 4], F32, name="bq")
            bk_sb = pers.tile([128, 4], F32, name="bk")
            bv_sb = pers.tile([1, DG], BF16, name="bv")
            rb_f = pers.tile([33, 512], F32, name="rb_f")
            rb_r = pers.tile([128, 512], F32R, name="rb_r")
            rbb = pers.tile([128, 512], F32, name="rbb")
            # row-selector weights for the denominator broadcast:
            # col block 0:64 selects row 0 (head A), 64:128 selects row 32 (B)
            onesel = pers.tile([128, 128], F32R, name="onesel")
            nc.sync.dma_start(out=onesel[:], in_=onesel_d[:])
            nc.sync.dma_start(out=rb_r[:], in_=rbones_d[:])

            nc.sync.dma_start(out=mask_sb[:], in_=mask01[:])
            nc.sync.dma_start(out=ones_bv[:], in_=ones_b[:])
            nc.sync.dma_start(out=bq_sb[:], in_=bq_c[:])
            nc.sync.dma_start(out=bk_sb[:], in_=bk_c[:])
            nc.sync.dma_start(out=bv_sb[:], in_=bv_r[:])
            nc.sync.dma_start(out=ones_w[:], in_=ones_col[:])

            # PSUM: sc 2x[128,1024]=4 banks, po 1, dn 1, pw 2 -> 8 banks
            pp = tc.alloc_tile_pool(name="pp", bufs=1, space="PSUM")

            # ---- phase 1: projections ----
            with (
                tc.tile_pool(name="wbig", bufs=3) as wp,
                tc.tile_pool(name="xqk", bufs=2) as xqkp,
                tc.tile_pool(name="xvp", bufs=3) as xvp,
            ):
                wq_big = wp.tile([128, 8 * 512], BF16, tag="wb", name="wq_big")
                wk_big = wp.tile([128, 8 * 512], BF16, tag="wb", name="wk_big")
                wv_big = wp.tile([128, 8 * 512], BF16, tag="wb", name="wv_big")
                nc.sync.dma_start(out=wv_big[:, 0:2048], in_=wv_c[:, 0:2048])
                nc.gpsimd.dma_start(out=wv_big[:, 2048:4096], in_=wv_c[:, 2048:4096])
                nc.gpsimd.dma_start(out=wq_big[:], in_=wq_c[:])
                nc.gpsimd.dma_start(out=wk_big[:], in_=wk_c[:])
                for c in range(4):
                    nc.gpsimd.dma_start(
                        out=wo_sb[c][:], in_=wo_c[c * 128 : (c + 1) * 128, :]
                    )

                # V: 16 s-tiles of 128
                for s in range(16):
                    xv_big = xvp.tile([128, 8 * 128], BF16, tag="xvb", name="xv_big")
                    nc.sync.dma_start(out=xv_big[:], in_=xv_c[s])
                    ps = pp.tile([128, DG], F32, tag="pw", bufs=2, name="psv")
                    for k8 in range(8):
                        nc.tensor.matmul(
                            ps[:],
                            xv_big[:, k8 * 128 : (k8 + 1) * 128],
                            wv_big[:, k8 * DG : (k8 + 1) * DG],
                            start=(k8 == 0),
                            stop=False,
                        )
                    nc.tensor.matmul(
                        ps[:], ones_bv[:], bv_sb[:], start=False, stop=True
                    )
                    nc.scalar.copy(vt[s][:], ps[:])

                # Q^T and K^T: two 1024-wide s-chunks
                for n2 in range(2):
                    xq_big = xqkp.tile([128, 8 * 1024], BF16, tag="xq", name="xq_big")
                    xk_big = xqkp.tile([128, 8 * 1024], BF16, tag="xk", name="xk_big")
                    nc.sync.dma_start(out=xq_big[:], in_=xq_c[n2])
                    nc.sync.dma_start(out=xk_big[:], in_=xk_c[n2])
                    for m in range(4):
                        for w_big, x_big, b_sb, dst, on_dve in (
                            (wq_big, xq_big, bq_sb, qT, True),
                            (wk_big, xk_big, bk_sb, kT, False),
                        ):
                            ps = pp.tile([128, 1024], F32, tag="sc", bufs=2, name="ps1")
                            for half in range(2):
                                for k8 in range(8):
                                    nc.tensor.matmul(
                                        ps[:, half * 512 : (half + 1) * 512],
                                        w_big[
                                            :,
                                            k8 * 512
                                            + m * 128 : k8 * 512
                                            + (m + 1) * 128,
                                        ],
                                        x_big[
                                            :,
                                            k8 * 1024
                                            + half * 512 : k8 * 1024
                                            + (half + 1) * 512,
                                        ],
                                        start=(k8 == 0),
                                        stop=(k8 == 7),
                                    )
                            dst_ap = dst[m][:, n2 * 1024 : (n2 + 1) * 1024]
                            if on_dve:
                                nc.vector.tensor_scalar_add(
                                    dst_ap, ps[:], b_sb[:, m : m + 1]
                                )
                            else:
                                nc.scalar.activation(
                                    dst_ap, ps[:], AF.Identity,
                                    bias=b_sb[:, m : m + 1],
                                )

            # ---- phase 2: causal attention, j (sq chunk) outer ----
            with (
                tc.tile_pool(name="at", bufs=3) as ap_,
                tc.tile_pool(name="ob", bufs=2) as obp,
            ):
                # Wo drip-feed: tasks gated on how many pairs of their
                # j-chunk have finished attention (pairs_done[j])
                wo_tasks = []
                wo_cur = None  # [psw, ob, j, s2, n2, mms_done]
                pairs_done = [0, 0, 0, 0]

                def wo_step():
                    nonlocal wo_cur
                    if wo_cur is None:
                        if not wo_tasks:
                            return False
                        wj, s2, n2, ob = wo_tasks[0]
                        if pairs_done[wj] == 0:
                            return False
                        wo_tasks.pop(0)
                        psw = pp.tile([128, 512], F32, tag="pw", bufs=2, name="psw")
                        wo_cur = [psw, ob, wj, s2, n2, 0]
                    psw, ob, wj, s2, n2, done = wo_cur
                    if done < 4:
                        if pairs_done[wj] <= done:
                            return False
                        nc.tensor.matmul(
                            psw[:],
                            aout[done][:, (wj * 4 + s2) * 128 : (wj * 4 + s2 + 1) * 128],
                            wo_sb[done][:, n2 * 512 : (n2 + 1) * 512],
                            start=(done == 0),
                            stop=(done == 3),
                        )
                        wo_cur[5] = done + 1
                    else:
                        ob3 = ob[:].rearrange("p (s2 n2 c) -> p s2 n2 c", s2=4, n2=2)
                        nc.vector.tensor_copy(ob3[:, s2, n2, :], psw[:])
                        if n2 == 1:
                            nc.sync.dma_start(
                                out=out_d[
                                    wj * 512 + s2 * 128 : wj * 512 + (s2 + 1) * 128, :
                                ],
                                in_=ob3[:, s2, :, :].rearrange("p n2 c -> p (n2 c)"),
                            )
                        wo_cur = None
                    return True

                pending_norm = None

                def norm_fin_make(p, j, ps_o, dn):
                    def fin():
                        # one full-array matmul: cols 0:64 of onesel select
                        # rb row 0 (head A), cols 64:128 select row 32 (B)
                        nc.tensor.matmul(
                            dn[:],
                            onesel[:],
                            rb_r[:],
                            start=True,
                            stop=True,
                        )
                        nc.vector.tensor_copy(rbb[:], dn[:])
                        nc.vector.tensor_mul(
                            aout[p][0:64, j * 512 : (j + 1) * 512],
                            ps_o[0:64, :],
                            rbb[0:64, :],
                        )
                        nc.vector.tensor_mul(
                            aout[p][64:128, j * 512 : (j + 1) * 512],
                            ps_o[64:128, :],
                            rbb[64:128, :],
                        )

                    return fin

                for j in range(4):
                    ob = obp.tile([128, 4096], BF16, tag="ob", name="ob")
                    for s2 in range(4):
                        for n2 in range(2):
                            wo_tasks.append((j, s2, n2, ob))
                    for p in range(4):
                        nsk = 4 * j + 4
                        ps_o = pp.tile([128, 512], F32, tag="po", bufs=1, name="ps_o")
                        dn = pp.tile([128, 512], F32, tag="dn", bufs=1, name="dn")
                        pending = None

                        def retire(last):
                            i, c0, at_r = pending
                            nc.tensor.matmul(
                                ps_o[0:64, c0:512],
                                vt[i][:, p * 128 : p * 128 + 64],
                                at_r[:, 0, c0:512],
                                start=(i == 0),
                                stop=last,
                                tile_position=(0, 0),
                            )
                            nc.tensor.matmul(
                                ps_o[64:128, c0:512],
                                vt[i][:, p * 128 + 64 : (p + 1) * 128],
                                at_r[:, 1, c0:512],
                                start=(i == 0),
                                stop=last,
                                tile_position=(0, 64),
                            )
                            nc.tensor.matmul(
                                dn[0:1, c0:512],
                                ones_w[:],
                                at_r[:, 0, c0:512],
                                start=(i == 0),
                                stop=last,
                                tile_position=(0, 0),
                            )
                            nc.tensor.matmul(
                                dn[32:33, c0:512],
                                ones_w[:],
                                at_r[:, 1, c0:512],
                                start=(i == 0),
                                stop=last,
                                tile_position=(0, 32),
                            )

                        for i in range(nsk):
                            koff = i - 4 * j
                            c0 = max(0, koff * 128)
                            ps_s = pp.tile(
                                [128, 1024], F32, tag="sc", bufs=2, name="ps_s"
                            )
                            ps3 = ps_s[:].rearrange("p (h c) -> p h c", h=2)
                            nc.tensor.matmul(
                                ps3[:, 0, c0:512],
                                kT[p][0:64, i * 128 : (i + 1) * 128],
                                qT[p][0:64, j * 512 + c0 : (j + 1) * 512],
                                start=True,
                                stop=True,
                                tile_position=(0, 0),
                            )
                            nc.tensor.matmul(
                                ps3[:, 1, c0:512],
                                kT[p][64:128, i * 128 : (i + 1) * 128],
                                qT[p][64:128, j * 512 + c0 : (j + 1) * 512],
                                start=True,
                                stop=True,
                                tile_position=(64, 0),
                            )
                            if pending is not None:
                                retire(False)
                            at = ap_.tile([128, 1024], BF16, tag="at", name="at")
                            at_r = at[:].rearrange("p (h c) -> p h c", h=2)
                            nc.scalar.activation(
                                at_r[:, :, c0:512],
                                ps3[:, :, c0:512],
                                AF.Exp,
                                scale=0.125,
                            )
                            if koff >= 0:
                                nc.vector.tensor_mul(
                                    at_r[:, 0, c0 : c0 + 128],
                                    at_r[:, 0, c0 : c0 + 128],
                                    mask_sb[:],
                                )
                                nc.vector.tensor_mul(
                                    at_r[:, 1, c0 : c0 + 128],
                                    at_r[:, 1, c0 : c0 + 128],
                                    mask_sb[:],
                                )
                            if pending_norm is not None:
                                pending_norm()
                                pending_norm = None
                            wo_step()
                            if j >= 2:
                                wo_step()
                            pending = (i, c0, at_r)
                        retire(True)
                        pending = None

                        # normalization: recip(den) -> f32r copy now; the PE
                        # broadcast + muls are deferred into the next chunk's
                        # first iteration so the PE never waits on this chain
                        nc.vector.reciprocal_approx_fast(rb_f[0:33, :], dn[0:33, :])
                        nc.vector.tensor_copy(rb_r[0:1, :], rb_f[0:1, :])
                        nc.vector.tensor_copy(rb_r[32:33, :], rb_f[32:33, :])
                        pending_norm = norm_fin_make(p, j, ps_o, dn)
                        pairs_done[j] = p + 1

                # flush deferred norm and remaining Wo work
                if pending_norm is not None:
                    pending_norm()
                    pending_norm = None
                while wo_step():
                    pass

            pp.release()

    if do_compile:
        nc.compile()
    return nc


def _make_in_maps(query, key, value, wq, bq, wk, bk, wv, bv, wo):
    f32 = np.float32
    ones_b = np.ones((1, 128), BF)
    ones_col = np.ones((128, 1), BF)
    onesel = np.zeros((128, 128), f32)
    onesel[0, 0:64] = 1.0
    onesel[32, 64:128] = 1.0
    rbones = np.ones((128, 512), f32)
    mask01 = np.triu(np.ones((128, 128), BF))

    wqT = np.asarray(wq, f32).T.astype(BF)  # [D, D] (d_in, d_out)
    wkT = np.asarray(wk, f32).T.astype(BF)
    wvT = np.asarray(wv, f32).T.astype(BF)
    woT = np.asarray(wo, f32).T.astype(BF)  # [dv, D]

    def warr(wT, sl):
        # [1024, 512] -> [p, k*512] with d_in = k*128 + p
        return np.ascontiguousarray(
            wT[:, sl].reshape(8, 128, 512).transpose(1, 0, 2).reshape(128, 8 * 512)
        )

    def xqk_arr(x):
        # x [S, D] -> xT [D, S]; element [k*128+p, n2*1024+t]
        # -> [n2, p, k*1024 + t]
        xT = np.asarray(x, f32).T.astype(BF)
        return np.ascontiguousarray(
            xT.reshape(8, 128, 2, 1024).transpose(2, 1, 0, 3).reshape(2, 128, 8 * 1024)
        )

    def xv_arr(x):
        # element [k*128+p, s*128+t] -> [s, p, k*128 + t]
        xT = np.asarray(x, f32).T.astype(BF)
        return np.ascontiguousarray(
            xT.reshape(8, 128, 16, 128).transpose(2, 1, 0, 3).reshape(16, 128, 8 * 128)
        )

    in_maps = []
    for c in range(8):
        b, g = c // 2, c % 2
        sl = slice(g * DG, (g + 1) * DG)
        in_maps.append(
            {
                "xq_c": xqk_arr(query[b]),
                "xk_c": xqk_arr(key[b]),
                "xv_c": xv_arr(value[b]),
                "wq_c": warr(wqT, sl),
                "wk_c": warr(wkT, sl),
                "wv_c": warr(wvT, sl),
                "wo_c": np.ascontiguousarray(woT[sl, :]),
                "bq_c": np.ascontiguousarray(
                    np.asarray(bq, f32)[sl].reshape(4, 128).T
                ),
                "bk_c": np.ascontiguousarray(
                    np.asarray(bk, f32)[sl].reshape(4, 128).T
                ),
                "bv_r": np.asarray(bv, f32)[sl].reshape(1, DG).astype(BF),
                "ones_b": ones_b,
                "ones_col": ones_col,
                "onesel_d": onesel,
                "rbones_d": rbones,
                "mask01": mask01,
            }
        )
    return in_maps


def kernel(query, key, value, mask, wq, bq, wk, bk, wv, bv, wo, bo):
    global _PROGRAM, LAST_RESULTS
    if _PROGRAM is None:
        _PROGRAM = _build_program()
    nc = _PROGRAM
    in_maps = _make_in_maps(query, key, value, wq, bq, wk, bk, wv, bv, wo)

    res = run_bass_kernel_spmd(nc, in_maps, core_ids=list(range(8)))
    LAST_RESULTS = res

    f32 = np.float32
    out = np.empty((B, S, D), f32)
    for b in range(B):
        out[b] = res.results[2 * b]["out0"].astype(f32) + res.results[2 * b + 1][
            "out0"
        ].astype(f32)
    out += np.asarray(bo, f32)[None, None, :]
    return out
